# revision 31
# baseline (speedup 1.0000x reference)
"""Embedding-bag kernel for Trainium2, 8 NeuronCores — v4.

Data-parallel: batch sharded 8 ways, tables replicated per core; each core
gathers its 512 rows' token embeddings (per-row token streams packed on the
host into int16 index streams over two 50k vocab chunks) with SWDGE
dma_gather, then reduces each 128-row block with a contiguous halving tree
on DVE and writes [128, 64] accs out.

Measured constraints that shaped this version (vs v3):
- dma_gather is capped at 1024 indices: the SWDGE descriptor ring holds only
  ~80 descriptors per (queue, engine) and a 1024-idx gather = 65/engine;
  1280+ hangs the device. Bigger rings are not host-configurable (the
  MEMCOPY_CARVEOUT_CFG register is runtime-programmed).
- Random 256 B HBM reads sustain only ~95-105 GB/s per core through the DMA
  engines (vs ~360 GB/s streaming), making the ~53 MB/core of gather payload
  an ~520 us floor; the kernel runs at ~97% of that wall.
- transpose/SBUF-source dma_gather modes hang on this hardware (verified by
  microbenchmark), ruling out SBUF-resident vocab-sharded designs.
- The DVE reduce is a fully contiguous halving tree (1 elem/cycle every
  pass, ~23% DVE busy), replacing v3's strided half-rate reduce; a single
  in-order `vdone` job counter recycles gather slots and gates output DMAs.
- Per-row token streams are sorted ascending by vocab index (marginal DRAM
  locality win, never worse).
- Block(no_gpsimd_drain=True): the block-exit dge_drain is redundant (all
  data deps are semaphore-gated) and skipping it trims the epilogue.
- 512-idx gathers retested with interleaved runs (device is bimodal
  ~545/~655 us): consistently ~50-60 us slower than 1024-idx in BOTH modes.
"""

import sys

if "/opt/trn_rl_repo" not in sys.path:
    sys.path.insert(0, "/opt/trn_rl_repo")

from contextlib import ExitStack

import numpy as np

import concourse.bacc as bacc
import concourse.bass as bass
import concourse.mybir as mybir
from concourse import library_config
from concourse.bass_utils import run_bass_kernel_spmd

N_CORES = 8
P = 128
VOCAB = 100000
SEQ = 200
DIM = 64
BATCH = 4096

N_CHUNKS = 2
CHUNK = VOCAB // N_CHUNKS          # 50000 (signed int16 indexing)
CHUNK_ROWS = CHUNK + 1             # + zero pad row
BASE_SHIFT = 17233                 # in_ap base shift; locals in [-17233, 32767]
PAD_IDX = CHUNK - BASE_SHIFT       # local index of the zero row (positive)
GMAX_COLS = 8                      # max dest columns per gather (1024 idxs)
NBUF = 5
NQ = 4
DMA_SCRATCH = 16384                # default SWDGE ring carveout


def _gather_plan(kj):
    """Split kj columns into gathers of <=GMAX_COLS columns."""
    sizes = [GMAX_COLS] * (kj // GMAX_COLS)
    if kj % GMAX_COLS:
        sizes.append(kj % GMAX_COLS)
    return sizes


def build_nc(K, n_blocks, idx_cols, split_col):
    """K: [2, N_CHUNKS, n_blocks] exact max counts (identical across cores).
    idx_cols: total int16 columns of gidx. split_col: boundary of the first
    idx DMA (jobs whose columns start past it wait for the second DMA)."""
    kmax = int(K.max())

    nc = bacc.Bacc("TRN2", debug=False, num_swdge_queues=NQ,
                   dynamic_dma_scratch_size=DMA_SCRATCH)

    emb_cat = nc.dram_tensor(
        "emb_cat", [2 * N_CHUNKS * CHUNK_ROWS, DIM], mybir.dt.float32,
        kind="ExternalInput",
    )
    gidx = nc.dram_tensor("gidx", [P, idx_cols], mybir.dt.int16, kind="ExternalInput")
    out_pri = nc.dram_tensor("out_pri", [n_blocks * P, DIM], mybir.dt.float32, kind="ExternalOutput")
    out_sec = nc.dram_tensor("out_sec", [n_blocks * P, DIM], mybir.dt.float32, kind="ExternalOutput")
    outs = (out_pri, out_sec)

    jobs = [(t, b, k) for t in range(2) for b in range(n_blocks) for k in range(N_CHUNKS)]

    with (
        nc.Block(no_gpsimd_drain=True) as _block,
        nc.sbuf_tensor("gidx_sb", [P, idx_cols], mybir.dt.int16) as gidx_sb,
        nc.semaphore("io") as io,
        ExitStack() as stack,
    ):
        slots = [
            stack.enter_context(
                nc.sbuf_tensor(f"slot{i}", [P, kmax * DIM], mybir.dt.float32)
            )
            for i in range(NBUF)
        ]
        accs = [
            stack.enter_context(
                nc.sbuf_tensor(f"acc{t}_{b}", [P, DIM], mybir.dt.float32)
            )
            for t in range(2)
            for b in range(n_blocks)
        ]
        done = [
            [stack.enter_context(nc.semaphore(f"done{i}_{q}")) for q in range(NQ)]
            for i in range(NBUF)
        ]
        # counts jobs fully reduced, in job order (DVE completes in order):
        # job j may reuse slot j%NBUF once vdone >= j-NBUF+1, and (t,b)'s acc
        # is ready once vdone passes its k==1 job.
        vdone = stack.enter_context(nc.semaphore("vdone"))

        # ---- sync engine: two-stage index load so gathers start early
        nc.sync.dma_start(gidx_sb[:, :split_col], gidx[:, :split_col]).then_inc(io, 16)
        nc.sync.dma_start(gidx_sb[:, split_col:], gidx[:, split_col:]).then_inc(io, 16)

        # ---- gpsimd: all gathers
        nc.gpsimd.load_library(library_config.mlp)
        nc.gpsimd.wait_ge(io, 16)
        waited_full = False
        gq = 0            # queue rotation counter
        icol = 0          # running int16 column offset into gidx_sb
        done_target = [[0] * NQ for _ in range(NBUF)]
        for j, (t, b, k) in enumerate(jobs):
            slot = j % NBUF
            if j >= NBUF:
                nc.gpsimd.wait_ge(vdone, j - NBUF + 1)
            kj = int(K[t, k, b])
            base = (t * N_CHUNKS + k) * CHUNK_ROWS + BASE_SHIFT
            src = emb_cat[base:(t * N_CHUNKS + k + 1) * CHUNK_ROWS, :]
            g3 = slots[slot][:].rearrange("p (c d) -> p c d", d=DIM)
            col = 0
            for size in _gather_plan(kj):
                nidx = size * P
                ic = nidx // 16
                if not waited_full and icol + ic > split_col:
                    nc.gpsimd.wait_ge(io, 32)
                    waited_full = True
                q = gq % NQ
                nc.gpsimd.dma_gather(
                    g3[:, col:col + size, :],
                    src,
                    gidx_sb[:, icol:icol + ic],
                    nidx,
                    nidx,
                    DIM,
                    queue_num=q,
                ).then_inc(done[slot][q], 16)
                done_target[slot][q] += 16
                gq += 1
                icol += ic
                col += size
            jobs[j] = (t, b, k, slot, tuple(done_target[slot]), kj)

        # ---- vector: contiguous halving tree, accumulate chunks, recycle slots
        for j, (t, b, k, slot, tgts, kj) in enumerate(jobs):
            for q in range(NQ):
                if tgts[q]:
                    nc.vector.wait_ge(done[slot][q], tgts[q])
            g = slots[slot]
            acc = accs[t * n_blocks + b]
            n = kj
            while n > 2:
                h = n // 2
                nc.vector.tensor_add(
                    out=g[:, : h * DIM],
                    in0=g[:, : h * DIM],
                    in1=g[:, (n - h) * DIM : n * DIM],
                )
                n -= h
            if k == 0:
                if n == 1:
                    fin = nc.vector.tensor_copy(out=acc[:], in_=g[:, :DIM])
                else:
                    fin = nc.vector.tensor_add(
                        out=acc[:], in0=g[:, :DIM], in1=g[:, DIM : 2 * DIM]
                    )
            else:
                if n == 2:
                    nc.vector.tensor_add(
                        out=g[:, :DIM], in0=g[:, :DIM], in1=g[:, DIM : 2 * DIM]
                    )
                fin = nc.vector.tensor_add(out=acc[:], in0=acc[:], in1=g[:, :DIM])
            fin.then_inc(vdone, 1)

        # ---- sync engine: write outputs as accs complete
        m = 0
        for t in range(2):
            for b in range(n_blocks):
                m += 1
                nc.sync.wait_ge(vdone, (t * n_blocks + b) * N_CHUNKS + 2)
                nc.sync.dma_start(
                    out=outs[t][b * P:(b + 1) * P, :],
                    in_=accs[t * n_blocks + b][:],
                ).then_inc(io, 16)
        nc.sync.wait_ge(io, 32 + m * 16)

    nc.compile()
    return nc


def _pack_core(idx_sorted, K, n_blocks, idx_cols):
    """idx_sorted: [2, bc, SEQ] row-sorted core indices. Returns gidx
    [128, idx_cols] int16: the full gather stream wrapped into 16 partitions
    and replicated 8x (all Q7 cpu bands see the whole stream). A disjoint
    per-queue-band packing (4x less idx traffic) measured ~15% SLOWER
    end-to-end, so the replicated layout stays."""
    streams = []
    for t in range(2):
        for b in range(n_blocks):
            rows = idx_sorted[t][b * P:(b + 1) * P]
            for k in range(N_CHUNKS):
                kj = int(K[t, k, b])
                mask = (rows // CHUNK) == k
                local = (rows - k * CHUNK - BASE_SHIFT).astype(np.int64)
                # in-chunk tokens first, ascending vocab order (DRAM locality)
                key = np.where(mask, local, np.int64(1) << 40)
                order = np.argsort(key, axis=1, kind="stable")
                sortloc = np.take_along_axis(local, order, axis=1)
                cnt = mask.sum(axis=1)
                pad_cols = max(kj - SEQ, 0)
                if pad_cols:
                    sortloc = np.concatenate(
                        [sortloc, np.zeros((P, pad_cols), np.int64)], axis=1
                    )
                sel = sortloc[:, :kj]
                sel = np.where(np.arange(kj)[None, :] < cnt[:, None], sel, PAD_IDX)
                # Every gather's final stream slot (lane 127, last column of
                # the gather) must be >= 0: ucode trims trailing negatives.
                row127 = sel[127].copy()
                lasts = []
                c = 0
                for size in _gather_plan(kj):
                    c += size
                    lasts.append(c - 1)
                lastset = set(lasts)
                for last in lasts:
                    if row127[last] < 0:
                        cand = [jj for jj in range(kj)
                                if row127[jj] >= 0 and jj not in lastset]
                        assert cand, "no non-negative index for lane 127"
                        jj = cand[0]
                        row127[last], row127[jj] = row127[jj], row127[last]
                sel[127] = row127
                # column-major stream, split per gather
                c = 0
                for size in _gather_plan(kj):
                    streams.append(sel[:, c:c + size].T.ravel())
                    c += size
    s = np.concatenate(streams).astype(np.int16)
    wrapped = s.reshape(-1, 16).T
    assert wrapped.shape[1] == idx_cols
    return np.tile(wrapped, (8, 1)).copy()


def kernel(inputs_pri, inputs_sec, emb_pri, emb_sec, _trace=False, _trace_kwargs=None):
    inputs_pri = np.ascontiguousarray(np.asarray(inputs_pri, dtype=np.int32))
    inputs_sec = np.ascontiguousarray(np.asarray(inputs_sec, dtype=np.int32))
    emb_pri = np.ascontiguousarray(np.asarray(emb_pri, dtype=np.float32))
    emb_sec = np.ascontiguousarray(np.asarray(emb_sec, dtype=np.float32))

    batch = inputs_pri.shape[0]
    bc = batch // N_CORES
    n_blocks = bc // P

    emb_cat = np.zeros((2, N_CHUNKS, CHUNK_ROWS, DIM), np.float32)
    for t, emb in enumerate((emb_pri, emb_sec)):
        for k in range(N_CHUNKS):
            emb_cat[t, k, :CHUNK] = emb[k * CHUNK:(k + 1) * CHUNK]
    emb_cat = np.ascontiguousarray(emb_cat.reshape(2 * N_CHUNKS * CHUNK_ROWS, DIM))

    # Global c0-sort + round-robin deal: core c takes globally-sorted rows
    # c, c+8, c+16, ... so every core's per-block count maxima are nearly
    # identical — the compiled K (shared across cores) stops paying a
    # cross-core max penalty (~2.5% of gather traffic). The deal is done
    # independently per table; rowids[c][t] maps core-row -> batch row.
    sorted_rows = []   # per core: [2][bc, SEQ]
    rowids = []        # per core: [2][bc] absolute batch indices
    K = np.zeros((2, N_CHUNKS, n_blocks), np.int64)
    galloc = []
    for t, full in enumerate((inputs_pri, inputs_sec)):
        c0_all = ((full // CHUNK) == 0).sum(axis=1)
        gorder = np.argsort(c0_all, kind="stable")
        galloc.append((c0_all, gorder))
    for c in range(N_CORES):
        rows_c, ids_c = [], []
        for t, full in enumerate((inputs_pri, inputs_sec)):
            c0_all, gorder = galloc[t]
            ids = gorder[c::N_CORES]          # c0-ascending within the core
            rows_c.append(full[ids])
            ids_c.append(ids)
            c0s = c0_all[ids]
            for b in range(n_blocks):
                blk = c0s[b * P:(b + 1) * P]
                K[t, 0, b] = max(K[t, 0, b], blk.max())
                K[t, 1, b] = max(K[t, 1, b], SEQ - blk.min())
        sorted_rows.append(rows_c)
        rowids.append(ids_c)
    K = np.maximum(K, 1)

    total_cols = int(K.sum())
    idx_cols = total_cols * P // 16
    # first DMA covers just job 0's columns so gathers start ASAP; the second
    # (bulk) DMA overlaps job 0's gathers
    split_col = max(64, (int(K[0, 0, 0]) * P // 16) // 64 * 64)
    split_col = min(split_col, idx_cols - 64)

    nc = build_nc(K, n_blocks, idx_cols, split_col)

    in_maps = []
    for c in range(N_CORES):
        gidx = _pack_core(sorted_rows[c], K, n_blocks, idx_cols)
        in_maps.append({"emb_cat": emb_cat, "gidx": gidx})

    kwargs = {}
    if _trace:
        kwargs["trace"] = True
        if _trace_kwargs:
            kwargs.update(_trace_kwargs)
    res = run_bass_kernel_spmd(nc, in_maps, list(range(N_CORES)), **kwargs)
    outs = res.results
    out_pri = np.empty((batch, DIM), np.float32)
    out_sec = np.empty((batch, DIM), np.float32)
    for c in range(N_CORES):
        for t, out_full in enumerate((out_pri, out_sec)):
            res_c = outs[c]["out_pri" if t == 0 else "out_sec"]
            out_full[rowids[c][t]] = res_c
    if _trace:
        return (out_pri, out_sec), res
    return out_pri, out_sec
